# revision 1
# baseline (speedup 1.0000x reference)
"""HAN (heterogeneous attention network) forward on 8 trn2 NeuronCores.

Strategy: shard destination nodes across the 8 cores (6250 each). Host
pre-sorts each core's incident edges per metapath by destination block
(128 dst nodes per block) and folds the symmetric GraphConv normalization
rsqrt(deg_out[src])*rsqrt(deg_in[dst]) into a per-edge weight. On device,
each 128-edge chunk is gathered from the replicated h via indirect DMA,
a [128e x 128v] selection matrix is built in one fused vector op
(iota == dst_local) * w, and one TensorE matmul per chunk accumulates the
block aggregate in PSUM (agg[v, f] += sel^T @ et); per block two PE
transposes produce the transposed aggregate for the weight matmul.
Per metapath the GraphConv weight is then applied as zT = W^T @ aggT (+b),
semantic attention scores are reduced locally, one tiny AllReduce combines
the per-metapath score sums across cores, and the softmax-weighted
combination is written back transposed; the host re-transposes and stitches.
"""

import numpy as np

import concourse.bass as bass
import concourse.mybir as mybir
import concourse.tile as tile
from concourse import bacc
from concourse.bass_utils import run_bass_kernel_spmd
from concourse.masks import make_identity

N, F, D, P, E, CORES, SEM_H = 50000, 256, 256, 4, 800000, 8, 128
NC = N // CORES            # 6250 dst nodes per core
NB = (NC + 127) // 128     # 49 blocks
NCP = NB * 128             # 6272 padded nodes per core
FH = F // 128              # 2 feature halves
DH = D // 128              # 2 output halves

_nc_cache = {}


def _n_tiles():
    tiles = []
    off = 0
    while off < NCP:
        t = min(512, NCP - off)
        tiles.append((off, t))
        off += t
    return tiles


def _build(K):
    CH = P * NB * K  # chunk columns per core
    nc = bacc.Bacc("TRN2", target_bir_lowering=False, debug=False,
                   num_devices=CORES)
    dt = mybir.dt
    h = nc.dram_tensor("h", [N, F], dt.float32, kind="ExternalInput").ap()
    src = nc.dram_tensor("src", [128, CH], dt.int32, kind="ExternalInput").ap()
    dstf = nc.dram_tensor("dstf", [128, CH], dt.float32, kind="ExternalInput").ap()
    wf = nc.dram_tensor("wf", [128, CH], dt.float32, kind="ExternalInput").ap()
    Wgc = nc.dram_tensor("Wgc", [P, F, D], dt.float32, kind="ExternalInput").ap()
    bgc = nc.dram_tensor("bgc", [128, P * DH], dt.float32, kind="ExternalInput").ap()
    W1 = nc.dram_tensor("W1", [D, SEM_H], dt.float32, kind="ExternalInput").ap()
    b1 = nc.dram_tensor("b1", [SEM_H, 1], dt.float32, kind="ExternalInput").ap()
    w2 = nc.dram_tensor("w2", [SEM_H, 1], dt.float32, kind="ExternalInput").ap()
    out = nc.dram_tensor("out", [D, NCP], dt.float32, kind="ExternalOutput").ap()

    ntiles = _n_tiles()

    with tile.TileContext(nc) as tc:
        with (
            tc.tile_pool(name="const", bufs=1) as cp,
            tc.tile_pool(name="stage", bufs=4) as stp,
            tc.tile_pool(name="edges", bufs=12) as ep,
            tc.tile_pool(name="sel", bufs=12) as selp,
            tc.tile_pool(name="work", bufs=4) as wp,
            tc.tile_pool(name="psum_agg", bufs=2, space="PSUM") as pa,
            tc.tile_pool(name="psum_mm", bufs=4, space="PSUM") as pm,
            tc.tile_pool(name="dram", bufs=2, space="DRAM") as dp,
        ):
            # ---- constants ----
            iota_i = cp.tile([128, 128], dt.int32)
            nc.gpsimd.iota(iota_i[:], pattern=[[1, 128]], base=0,
                           channel_multiplier=0)
            iota_f = cp.tile([128, 128], dt.float32)
            nc.vector.tensor_copy(out=iota_f[:], in_=iota_i[:])
            ident = cp.tile([128, 128], dt.float32)
            make_identity(nc, ident[:])
            ones1 = cp.tile([1, 128], dt.float32)
            nc.vector.memset(ones1[:], 1.0)

            wgc_sb = []
            for p in range(P):
                per_fh = []
                for fh in range(FH):
                    t32 = stp.tile([128, D], dt.float32)
                    nc.sync.dma_start(out=t32[:], in_=Wgc[p, fh * 128:(fh + 1) * 128, :])
                    t16 = cp.tile([128, D], dt.bfloat16, name=f"wgc{p}_{fh}")
                    nc.vector.tensor_copy(out=t16[:], in_=t32[:])
                    per_fh.append(t16)
                wgc_sb.append(per_fh)
            bgc_sb = cp.tile([128, P * DH], dt.float32)
            nc.sync.dma_start(out=bgc_sb[:], in_=bgc[:])
            w1_sb = []
            for dh in range(DH):
                t32 = stp.tile([128, SEM_H], dt.float32)
                nc.sync.dma_start(out=t32[:], in_=W1[dh * 128:(dh + 1) * 128, :])
                t16 = cp.tile([128, SEM_H], dt.bfloat16, name=f"w1_{dh}")
                nc.vector.tensor_copy(out=t16[:], in_=t32[:])
                w1_sb.append(t16)
            b1_sb = cp.tile([128, 1], dt.float32)
            nc.sync.dma_start(out=b1_sb[:], in_=b1[:])
            w2_32 = stp.tile([128, 1], dt.float32)
            nc.sync.dma_start(out=w2_32[:], in_=w2[:])
            w2_sb = cp.tile([128, 1], dt.bfloat16)
            nc.vector.tensor_copy(out=w2_sb[:], in_=w2_32[:])

            aggT_sb = cp.tile([128, FH * NCP], dt.bfloat16)
            zT_sb = [cp.tile([128, DH * NCP], dt.bfloat16, name=f"zT{p}")
                     for p in range(P)]
            s4_sb = cp.tile([1, 128], dt.float32)
            nc.vector.memset(s4_sb[:], 0.0)

            # ---- main: aggregation + per-metapath transform ----
            for p in range(P):
                for b in range(NB):
                    q0 = (p * NB + b) * K
                    src_sb = stp.tile([128, K], dt.int32)
                    nc.sync.dma_start(out=src_sb[:], in_=src[:, q0:q0 + K])
                    dst_sb = stp.tile([128, K], dt.float32)
                    nc.sync.dma_start(out=dst_sb[:], in_=dstf[:, q0:q0 + K])
                    w_sb = stp.tile([128, K], dt.float32)
                    nc.sync.dma_start(out=w_sb[:], in_=wf[:, q0:q0 + K])

                    acc = pa.tile([128, F], dt.float32, name="acc")
                    for k in range(K):
                        et = ep.tile([128, F], dt.float32)
                        nc.gpsimd.indirect_dma_start(
                            out=et[:], out_offset=None, in_=h[:],
                            in_offset=bass.IndirectOffsetOnAxis(
                                ap=src_sb[:, k:k + 1], axis=0))
                        sel = selp.tile([128, 128], dt.float32)
                        nc.vector.tensor_scalar(
                            out=sel[:], in0=iota_f[:],
                            scalar1=dst_sb[:, k:k + 1],
                            scalar2=w_sb[:, k:k + 1],
                            op0=mybir.AluOpType.is_equal,
                            op1=mybir.AluOpType.mult)
                        # acc[v, f] += sel.T @ et
                        nc.tensor.matmul(out=acc[:], lhsT=sel[:], rhs=et[:],
                                         start=(k == 0), stop=(k == K - 1))
                    agg_tmp = wp.tile([128, F], dt.float32, tag="aggtmp")
                    nc.scalar.activation(
                        out=agg_tmp[:], in_=acc[:],
                        func=mybir.ActivationFunctionType.Copy)
                    for fh in range(FH):
                        tp_ps = pa.tile([128, 128], dt.float32, name="tp_ps")
                        nc.tensor.transpose(
                            out=tp_ps[:],
                            in_=agg_tmp[:, fh * 128:(fh + 1) * 128],
                            identity=ident[:])
                        nc.scalar.activation(
                            out=aggT_sb[:, fh * NCP + b * 128:
                                        fh * NCP + (b + 1) * 128],
                            in_=tp_ps[:],
                            func=mybir.ActivationFunctionType.Copy)

                # zT = W^T @ aggT + b
                for dh in range(DH):
                    for (n0, nt) in ntiles:
                        zp = pm.tile([128, 512], dt.float32, tag="mm")
                        for fh in range(FH):
                            nc.tensor.matmul(
                                out=zp[:, :nt],
                                lhsT=wgc_sb[p][fh][:, dh * 128:(dh + 1) * 128],
                                rhs=aggT_sb[:, fh * NCP + n0:fh * NCP + n0 + nt],
                                start=(fh == 0), stop=(fh == FH - 1))
                        nc.vector.tensor_scalar(
                            out=zT_sb[p][:, dh * NCP + n0:dh * NCP + n0 + nt],
                            in0=zp[:, :nt],
                            scalar1=bgc_sb[:, p * DH + dh:p * DH + dh + 1],
                            scalar2=None,
                            op0=mybir.AluOpType.add)

                # semantic attention scores: s = tanh(z@W1+b1) @ w2
                for (n0, nt) in ntiles:
                    tp = pm.tile([128, 512], dt.float32, tag="mm")
                    for dh in range(DH):
                        nc.tensor.matmul(
                            out=tp[:, :nt],
                            lhsT=w1_sb[dh][:],
                            rhs=zT_sb[p][:, dh * NCP + n0:dh * NCP + n0 + nt],
                            start=(dh == 0), stop=(dh == DH - 1))
                    t_sb = wp.tile([128, 512], dt.bfloat16)
                    nc.scalar.activation(
                        out=t_sb[:, :nt], in_=tp[:, :nt],
                        func=mybir.ActivationFunctionType.Tanh,
                        bias=b1_sb[:, 0:1])
                    sp = pm.tile([1, 512], dt.float32, tag="mm")
                    nc.tensor.matmul(out=sp[:, :nt], lhsT=w2_sb[:],
                                     rhs=t_sb[:, :nt], start=True, stop=True)
                    # accumulate the per-node scores (real nodes only) into
                    # the per-metapath sum
                    nt_real = min(nt, NC - n0)
                    if nt_real > 0:
                        stmp = wp.tile([1, 1], dt.float32)
                        nc.vector.tensor_reduce(
                            out=stmp[:], in_=sp[:, :nt_real],
                            axis=mybir.AxisListType.X, op=mybir.AluOpType.add)
                        nc.vector.tensor_tensor(
                            out=s4_sb[:, p:p + 1], in0=s4_sb[:, p:p + 1],
                            in1=stmp[:], op=mybir.AluOpType.add)

            # ---- semantic softmax over metapaths (global mean via AllReduce) --
            cc_in = dp.tile([1, 128], dt.float32)
            cc_out = dp.tile([1, 128], dt.float32)
            nc.sync.dma_start(out=cc_in[:], in_=s4_sb[:])
            nc.gpsimd.collective_compute(
                "AllReduce", mybir.AluOpType.add,
                replica_groups=[list(range(CORES))],
                ins=[cc_in.opt()], outs=[cc_out.opt()])
            sall = wp.tile([1, 128], dt.float32)
            nc.sync.dma_start(out=sall[:], in_=cc_out[:])

            bexp = wp.tile([1, P], dt.float32)
            nc.scalar.activation(out=bexp[:], in_=sall[:, :P],
                                 func=mybir.ActivationFunctionType.Exp,
                                 scale=1.0 / N)
            bsum = wp.tile([1, 1], dt.float32)
            nc.vector.tensor_reduce(out=bsum[:], in_=bexp[:],
                                    axis=mybir.AxisListType.X,
                                    op=mybir.AluOpType.add)
            binv = wp.tile([1, 1], dt.float32)
            nc.vector.reciprocal(out=binv[:], in_=bsum[:])
            bnorm = wp.tile([1, P], dt.float32)
            nc.vector.tensor_scalar_mul(out=bnorm[:], in0=bexp[:],
                                        scalar1=binv[:, 0:1])
            bb_ps = pm.tile([128, P], dt.float32, tag="mm")
            nc.tensor.matmul(out=bb_ps[:], lhsT=ones1[:], rhs=bnorm[:],
                             start=True, stop=True)
            bb_sb = wp.tile([128, P], dt.float32)
            nc.vector.tensor_copy(out=bb_sb[:], in_=bb_ps[:])
            diag = []
            for p in range(P):
                dg = cp.tile([128, 128], dt.bfloat16, name=f"diag{p}")
                nc.vector.tensor_scalar_mul(out=dg[:], in0=ident[:],
                                            scalar1=bb_sb[:, p:p + 1])
                diag.append(dg)

            # ---- weighted combine + output ----
            for dh in range(DH):
                for (n0, nt) in ntiles:
                    op_ps = pm.tile([128, 512], dt.float32, tag="mm")
                    for p in range(P):
                        nc.tensor.matmul(
                            out=op_ps[:, :nt], lhsT=diag[p][:],
                            rhs=zT_sb[p][:, dh * NCP + n0:dh * NCP + n0 + nt],
                            start=(p == 0), stop=(p == P - 1))
                    ot = wp.tile([128, 512], dt.float32)
                    nc.vector.tensor_copy(out=ot[:, :nt], in_=op_ps[:, :nt])
                    nc.sync.dma_start(
                        out=out[dh * 128:(dh + 1) * 128, n0:n0 + nt],
                        in_=ot[:, :nt])
    nc.compile()
    return nc


def _balance(deg, caps):
    """Assign NC nodes to NB blocks, balancing all P per-metapath in-degree
    sums simultaneously (greedy, heaviest node first). deg: [P, NC].
    Returns (assign [NC], max block load)."""
    order = np.argsort(-deg.sum(axis=0), kind="stable")
    loads = np.zeros((NB, deg.shape[0]), dtype=np.int64)
    counts = np.zeros(NB, dtype=np.int64)
    assign = np.empty(NC, dtype=np.int64)
    for n in order:
        feas = counts < caps
        newmax = np.where(feas[:, None], loads + deg[:, n], 1 << 40).max(axis=1)
        b = int(np.argmin(newmax))
        assign[n] = b
        loads[b] += deg[:, n]
        counts[b] += 1
    return assign, int(loads.max())


def _prep_core(src_p, dst_p, w_p, base, K, blk_of, pos_of):
    """Per-core, per-metapath padded chunk arrays. Returns [NB*K, 128] arrays."""
    m = (dst_p >= base) & (dst_p < base + NC)
    s, d, w = src_p[m], dst_p[m] - base, w_p[m]
    blk = blk_of[d]
    order = np.argsort(blk, kind="stable")
    s, d, w, blk = s[order], d[order], w[order], blk[order]
    cnt = np.bincount(blk, minlength=NB)
    start = np.concatenate([[0], np.cumsum(cnt)])[:-1]
    pos = np.arange(len(d)) - start[blk]
    slot = blk * (K * 128) + pos
    si = np.zeros(NB * K * 128, dtype=np.int32)
    df = np.zeros(NB * K * 128, dtype=np.float32)
    wf = np.zeros(NB * K * 128, dtype=np.float32)
    si[slot] = s
    df[slot] = pos_of[d]
    wf[slot] = w
    return (si.reshape(NB * K, 128), df.reshape(NB * K, 128),
            wf.reshape(NB * K, 128))


def kernel(h, src, dst, W_gc, b_gc, W1, b1, w2):
    h = np.ascontiguousarray(h, dtype=np.float32)
    src = np.asarray(src)
    dst = np.asarray(dst)

    # per-metapath symmetric normalization folded into per-edge weights
    w_edge = []
    for p in range(P):
        deg_out = np.clip(np.bincount(src[p], minlength=N), 1, None)
        deg_in = np.clip(np.bincount(dst[p], minlength=N), 1, None)
        w_edge.append((1.0 / np.sqrt(deg_out[src[p]]) /
                       np.sqrt(deg_in[dst[p]])).astype(np.float32))

    # Balance nodes into blocks per core (all metapaths at once) so the max
    # edges-per-block — and hence K, the uniform chunks-per-block — is minimal.
    # The 22 pad slots stay at the tail of the last block (caps 48x128 + 106),
    # keeping real nodes in slots [0, NC) for the on-device score masking.
    caps = np.full(NB, 128, dtype=np.int64)
    caps[NB - 1] = NC - (NB - 1) * 128
    blk_of, pos_of, max_cnt = [], [], 0
    for c in range(CORES):
        base = c * NC
        deg = np.stack([
            np.bincount(dst[p][(dst[p] >= base) & (dst[p] < base + NC)] - base,
                        minlength=NC) for p in range(P)])
        assign, mx = _balance(deg, caps)
        max_cnt = max(max_cnt, mx)
        order = np.argsort(assign, kind="stable")
        pos = np.empty(NC, dtype=np.int64)
        starts = np.concatenate([[0], np.cumsum(np.bincount(assign,
                                                            minlength=NB))])
        pos[order] = np.arange(NC) - starts[assign[order]]
        blk_of.append(assign)
        pos_of.append(pos.astype(np.float32))
    K = (max_cnt + 127) // 128

    if K not in _nc_cache:
        _nc_cache[K] = _build(K)
    nc = _nc_cache[K]

    bgc_arr = np.zeros((128, P * DH), dtype=np.float32)
    for p in range(P):
        for dh in range(DH):
            bgc_arr[:, p * DH + dh] = b_gc[p, dh * 128:(dh + 1) * 128]

    in_maps = []
    for c in range(CORES):
        base = c * NC
        sis, dfs, wfs = [], [], []
        for p in range(P):
            si, df, wf = _prep_core(src[p], dst[p], w_edge[p], base, K,
                                    blk_of[c], pos_of[c])
            sis.append(si)
            dfs.append(df)
            wfs.append(wf)
        in_maps.append({
            "h": h,
            "src": np.concatenate(sis, axis=0).T.copy(),
            "dstf": np.concatenate(dfs, axis=0).T.copy(),
            "wf": np.concatenate(wfs, axis=0).T.copy(),
            "Wgc": np.ascontiguousarray(W_gc, dtype=np.float32),
            "bgc": bgc_arr,
            "W1": np.ascontiguousarray(W1, dtype=np.float32),
            "b1": np.asarray(b1, dtype=np.float32).reshape(SEM_H, 1),
            "w2": np.asarray(w2, dtype=np.float32).reshape(SEM_H, 1),
        })

    global _last_in_maps
    _last_in_maps = in_maps
    res = run_bass_kernel_spmd(nc, in_maps, list(range(CORES))).results
    out = np.empty((N, D), dtype=np.float32)
    for c in range(CORES):
        slot = blk_of[c] * 128 + pos_of[c].astype(np.int64)
        out[c * NC:(c + 1) * NC] = res[c]["out"][:, slot].T
    return out



# revision 20
# speedup vs baseline: 2.9221x; 2.9221x over previous
"""HAN (heterogeneous attention network) forward on 8 trn2 NeuronCores.

Strategy: shard destination nodes across the 8 cores (6250 each). Host
pre-sorts each core's incident edges per metapath by destination block
(128 dst nodes per block). The symmetric GraphConv normalization is
separable, w = rsqrt(deg_out)[src] * rsqrt(deg_in)[dst]: the src factor
is folded host-side into per-metapath bf16 copies of h, the dst factor
into the per-partition scale of the PSUM->SBUF aggregate copy on the
Scalar engine. h copies are split row-wise into h_lo[:32768]/h_hi so
edge source indices fit the int16 index space of the GPSIMD dma_gather
instruction. Per block the K*128 incident edge rows are fetched with TWO
dma_gather calls (lo+hi segments, padded with dst=200 edges that match
no node); descriptor generation is spread over the 4 SWDGE queues
(~4x parallel). Selection matrices are pure is_equal tensor_tensor ops
(4 chunks per op, broadcast APs) - tensor_tensor never enters the DVE
2-port perf mode that mutually excludes SWDGE descriptor generation.
One bf16 TensorE matmul per 128-edge chunk accumulates the block
aggregate in PSUM (agg[v, f] += sel^T @ et); per block two PE transposes
produce the transposed aggregate.
Per metapath the GraphConv weight is applied as zT = W^T @ aggT (+b),
semantic attention scores are reduced locally, one tiny AllReduce
combines the per-metapath score sums across cores, and the
softmax-weighted combination is written back transposed; the host
re-transposes and stitches.
"""

import ml_dtypes
import numpy as np

import concourse.bass as bass
import concourse.mybir as mybir
import concourse.tile as tile
from concourse import bacc
from concourse.bass_utils import run_bass_kernel_spmd
from concourse.masks import make_identity

N, F, D, P, E, CORES, SEM_H = 50000, 256, 256, 4, 800000, 8, 128
NC = N // CORES            # 6250 dst nodes per core
NB = (NC + 127) // 128     # 49 blocks
NCP = NB * 128             # 6272 padded nodes per core
FH = F // 128              # 2 feature halves
DH = D // 128              # 2 output halves
HLO = 32768                # rows in h_lo (int16-addressable)
HHI = N - HLO              # rows in h_hi

_nc_cache = {}


def _n_tiles():
    tiles = []
    off = 0
    while off < NCP:
        t = min(512, NCP - off)
        tiles.append((off, t))
        off += t
    return tiles


def _build(KLO, KHI):
    K = KLO + KHI              # chunks (of 128 edges) per block
    IC = K * 8                 # idx cols per block (int16)
    nc = bacc.Bacc("TRN2", target_bir_lowering=False, debug=False,
                   num_devices=CORES, num_swdge_queues=4)
    dt = mybir.dt
    h_lo = [nc.dram_tensor(f"hlo{p}", [HLO, F], dt.bfloat16,
                           kind="ExternalInput").ap() for p in range(P)]
    h_hi = [nc.dram_tensor(f"hhi{p}", [HHI, F], dt.bfloat16,
                           kind="ExternalInput").ap() for p in range(P)]
    gidx = nc.dram_tensor("gidx", [128, P * NB * IC], dt.int16,
                          kind="ExternalInput").ap()
    dstf = nc.dram_tensor("dstf", [128, P * NB * K], dt.bfloat16,
                          kind="ExternalInput").ap()
    bvec = nc.dram_tensor("bvec", [128, P * NB], dt.float32,
                          kind="ExternalInput").ap()
    Wgc = nc.dram_tensor("Wgc", [P, F, D], dt.float32, kind="ExternalInput").ap()
    bgc = nc.dram_tensor("bgc", [128, P * DH], dt.float32,
                         kind="ExternalInput").ap()
    W1 = nc.dram_tensor("W1", [D, SEM_H], dt.float32, kind="ExternalInput").ap()
    b1 = nc.dram_tensor("b1", [SEM_H, 1], dt.float32, kind="ExternalInput").ap()
    w2 = nc.dram_tensor("w2", [SEM_H, 1], dt.float32, kind="ExternalInput").ap()
    out = nc.dram_tensor("out", [D, NCP], dt.float32, kind="ExternalOutput").ap()

    ntiles = _n_tiles()

    with tile.TileContext(nc) as tc:
        with (
            tc.tile_pool(name="const", bufs=1) as cp,
            tc.tile_pool(name="stage", bufs=4) as stp,
            tc.tile_pool(name="meta", bufs=1) as mp,
            tc.tile_pool(name="edges", bufs=3) as ep,
            tc.tile_pool(name="sel", bufs=6) as selp,
            tc.tile_pool(name="work", bufs=3) as wp,
            tc.tile_pool(name="psum_agg", bufs=2, space="PSUM") as pa,
            tc.tile_pool(name="psum_mm", bufs=4, space="PSUM") as pm,
            tc.tile_pool(name="dram", bufs=2, space="DRAM") as dp,
        ):
            # ---- constants ----
            iota_i = cp.tile([128, 128], dt.int32)
            nc.gpsimd.iota(iota_i[:], pattern=[[1, 128]], base=0,
                           channel_multiplier=0)
            iota_f = cp.tile([128, 128], dt.bfloat16)
            nc.vector.tensor_copy(out=iota_f[:], in_=iota_i[:])
            ident = cp.tile([128, 128], dt.float32)
            make_identity(nc, ident[:])
            ones1 = cp.tile([1, 128], dt.float32)
            nc.vector.memset(ones1[:], 1.0)

            wgc_sb = []
            for p in range(P):
                per_fh = []
                for fh in range(FH):
                    t32 = stp.tile([128, D], dt.float32)
                    nc.sync.dma_start(out=t32[:], in_=Wgc[p, fh * 128:(fh + 1) * 128, :])
                    t16 = cp.tile([128, D], dt.bfloat16, name=f"wgc{p}_{fh}")
                    nc.vector.tensor_copy(out=t16[:], in_=t32[:])
                    per_fh.append(t16)
                wgc_sb.append(per_fh)
            bgc_sb = cp.tile([128, P * DH], dt.float32)
            nc.sync.dma_start(out=bgc_sb[:], in_=bgc[:])
            bv_sb = cp.tile([128, P * NB], dt.float32)
            nc.sync.dma_start(out=bv_sb[:], in_=bvec[:])
            w1_sb = []
            for dh in range(DH):
                t32 = stp.tile([128, SEM_H], dt.float32)
                nc.sync.dma_start(out=t32[:], in_=W1[dh * 128:(dh + 1) * 128, :])
                t16 = cp.tile([128, SEM_H], dt.bfloat16, name=f"w1_{dh}")
                nc.vector.tensor_copy(out=t16[:], in_=t32[:])
                w1_sb.append(t16)
            b1_sb = cp.tile([128, 1], dt.float32)
            nc.sync.dma_start(out=b1_sb[:], in_=b1[:])
            w2_32 = stp.tile([128, 1], dt.float32)
            nc.sync.dma_start(out=w2_32[:], in_=w2[:])
            w2_sb = cp.tile([128, 1], dt.bfloat16)
            nc.vector.tensor_copy(out=w2_sb[:], in_=w2_32[:])

            aggT_sb = cp.tile([128, FH * NCP], dt.bfloat16)
            zT_sb = [cp.tile([128, DH * NCP], dt.bfloat16, name=f"zT{p}")
                     for p in range(P)]
            s4_sb = cp.tile([1, 128], dt.float32)
            nc.vector.memset(s4_sb[:], 0.0)

            # ---- main: aggregation + per-metapath transform ----
            qn = 0
            for p in range(P):
                # preload this metapath's edge indices and dst positions
                idx_mp = mp.tile([128, NB * IC], dt.int16, tag="idx")
                nc.sync.dma_start(out=idx_mp[:],
                                  in_=gidx[:, p * NB * IC:(p + 1) * NB * IC])
                dst_mp = mp.tile([128, NB * K], dt.bfloat16, tag="dst")
                nc.sync.dma_start(out=dst_mp[:],
                                  in_=dstf[:, p * NB * K:(p + 1) * NB * K])
                for b in range(NB):
                    i0 = b * IC
                    # two gathers fetch the whole block's K*128 edge rows:
                    # et[q, g, :] = h[idx[g*128 + q]]; descriptor generation
                    # is parallelized across the 4 SWDGE queues
                    et = ep.tile([128, K * F], dt.bfloat16)
                    et3 = et[:].rearrange("q (g f) -> q g f", f=F)
                    nc.gpsimd.dma_gather(
                        et3[:, 0:KLO, :], h_lo[p][:],
                        idx_mp[:, i0:i0 + KLO * 8],
                        KLO * 128, KLO * 128, F, single_packet=False,
                        queue_num=qn)
                    qn = (qn + 1) % 4
                    nc.gpsimd.dma_gather(
                        et3[:, KLO:K, :], h_hi[p][:],
                        idx_mp[:, i0 + KLO * 8:i0 + IC],
                        KHI * 128, KHI * 128, F, single_packet=False,
                        queue_num=qn)
                    qn = (qn + 1) % 4

                    # selection matrices, 4 chunks per DVE op; pure is_equal
                    # tensor_tensor (never enters the DVE 2-port perf mode
                    # that starves SWDGE desc-gen); pad edges have dst=200
                    # which matches no lane of the 0..127 iota
                    sels = []
                    for g0 in range(0, K, 4):
                        gs = min(4, K - g0)
                        eq = selp.tile([128, 4 * 128], dt.bfloat16, tag="eq")
                        nc.vector.tensor_tensor(
                            out=eq[:, :gs * 128].rearrange(
                                "q (g v) -> q g v", v=128),
                            in0=iota_f[:].unsqueeze(1)
                                .broadcast_to([128, gs, 128]),
                            in1=dst_mp[:, b * K + g0:b * K + g0 + gs]
                                .unsqueeze(2).broadcast_to([128, gs, 128]),
                            op=mybir.AluOpType.is_equal)
                        sels.append(eq)
                    acc = pa.tile([128, F], dt.float32, name="acc")
                    for k in range(K):
                        sel = sels[k // 4][:, (k % 4) * 128:(k % 4 + 1) * 128]
                        # acc[v, f] += sel.T @ et
                        nc.tensor.matmul(out=acc[:], lhsT=sel,
                                         rhs=et[:, k * F:(k + 1) * F],
                                         start=(k == 0), stop=(k == K - 1))
                    # dst-side normalization folded into the PSUM->SBUF copy
                    agg_tmp = wp.tile([128, F], dt.float32, tag="aggtmp")
                    nc.scalar.activation(
                        out=agg_tmp[:], in_=acc[:],
                        func=mybir.ActivationFunctionType.Copy,
                        scale=bv_sb[:, p * NB + b:p * NB + b + 1])
                    for fh in range(FH):
                        tp_ps = pa.tile([128, 128], dt.float32, name="tp_ps")
                        nc.tensor.transpose(
                            out=tp_ps[:],
                            in_=agg_tmp[:, fh * 128:(fh + 1) * 128],
                            identity=ident[:])
                        nc.scalar.activation(
                            out=aggT_sb[:, fh * NCP + b * 128:
                                        fh * NCP + (b + 1) * 128],
                            in_=tp_ps[:],
                            func=mybir.ActivationFunctionType.Copy)

                # zT = W^T @ aggT + b
                for dh in range(DH):
                    for (n0, nt) in ntiles:
                        zp = pm.tile([128, 512], dt.float32, tag="mm")
                        for fh in range(FH):
                            nc.tensor.matmul(
                                out=zp[:, :nt],
                                lhsT=wgc_sb[p][fh][:, dh * 128:(dh + 1) * 128],
                                rhs=aggT_sb[:, fh * NCP + n0:fh * NCP + n0 + nt],
                                start=(fh == 0), stop=(fh == FH - 1))
                        nc.vector.tensor_tensor(
                            out=zT_sb[p][:, dh * NCP + n0:dh * NCP + n0 + nt],
                            in0=zp[:, :nt],
                            in1=bgc_sb[:, p * DH + dh:p * DH + dh + 1]
                                .to_broadcast([128, nt]),
                            op=mybir.AluOpType.add)

                # semantic attention scores: s = tanh(z@W1+b1) @ w2
                for (n0, nt) in ntiles:
                    tp = pm.tile([128, 512], dt.float32, tag="mm")
                    for dh in range(DH):
                        nc.tensor.matmul(
                            out=tp[:, :nt],
                            lhsT=w1_sb[dh][:],
                            rhs=zT_sb[p][:, dh * NCP + n0:dh * NCP + n0 + nt],
                            start=(dh == 0), stop=(dh == DH - 1))
                    t_sb = wp.tile([128, 512], dt.bfloat16)
                    nc.scalar.activation(
                        out=t_sb[:, :nt], in_=tp[:, :nt],
                        func=mybir.ActivationFunctionType.Tanh,
                        bias=b1_sb[:, 0:1])
                    sp = pm.tile([1, 512], dt.float32, tag="mm")
                    nc.tensor.matmul(out=sp[:, :nt], lhsT=w2_sb[:],
                                     rhs=t_sb[:, :nt], start=True, stop=True)
                    # accumulate the per-node scores (real nodes only) into
                    # the per-metapath sum
                    nt_real = min(nt, NC - n0)
                    if nt_real > 0:
                        stmp = wp.tile([1, 1], dt.float32)
                        nc.vector.tensor_reduce(
                            out=stmp[:], in_=sp[:, :nt_real],
                            axis=mybir.AxisListType.X, op=mybir.AluOpType.add)
                        nc.vector.tensor_tensor(
                            out=s4_sb[:, p:p + 1], in0=s4_sb[:, p:p + 1],
                            in1=stmp[:], op=mybir.AluOpType.add)

            # ---- semantic softmax over metapaths (global mean via AllReduce) --
            cc_in = dp.tile([1, 128], dt.float32)
            cc_out = dp.tile([1, 128], dt.float32)
            nc.sync.dma_start(out=cc_in[:], in_=s4_sb[:])
            nc.gpsimd.collective_compute(
                "AllReduce", mybir.AluOpType.add,
                replica_groups=[list(range(CORES))],
                ins=[cc_in.opt()], outs=[cc_out.opt()])
            sall = wp.tile([1, 128], dt.float32)
            nc.sync.dma_start(out=sall[:], in_=cc_out[:])

            bexp = wp.tile([1, P], dt.float32)
            nc.scalar.activation(out=bexp[:], in_=sall[:, :P],
                                 func=mybir.ActivationFunctionType.Exp,
                                 scale=1.0 / N)
            bsum = wp.tile([1, 1], dt.float32)
            nc.vector.tensor_reduce(out=bsum[:], in_=bexp[:],
                                    axis=mybir.AxisListType.X,
                                    op=mybir.AluOpType.add)
            binv = wp.tile([1, 1], dt.float32)
            nc.vector.reciprocal(out=binv[:], in_=bsum[:])
            bnorm = wp.tile([1, P], dt.float32)
            nc.vector.tensor_scalar_mul(out=bnorm[:], in0=bexp[:],
                                        scalar1=binv[:, 0:1])
            bb_ps = pm.tile([128, P], dt.float32, tag="mm")
            nc.tensor.matmul(out=bb_ps[:], lhsT=ones1[:], rhs=bnorm[:],
                             start=True, stop=True)
            bb_sb = wp.tile([128, P], dt.float32)
            nc.vector.tensor_copy(out=bb_sb[:], in_=bb_ps[:])
            diag = []
            for p in range(P):
                dg = cp.tile([128, 128], dt.bfloat16, name=f"diag{p}")
                nc.vector.tensor_scalar_mul(out=dg[:], in0=ident[:],
                                            scalar1=bb_sb[:, p:p + 1])
                diag.append(dg)

            # ---- weighted combine + output ----
            for dh in range(DH):
                for (n0, nt) in ntiles:
                    op_ps = pm.tile([128, 512], dt.float32, tag="mm")
                    for p in range(P):
                        nc.tensor.matmul(
                            out=op_ps[:, :nt], lhsT=diag[p][:],
                            rhs=zT_sb[p][:, dh * NCP + n0:dh * NCP + n0 + nt],
                            start=(p == 0), stop=(p == P - 1))
                    ot = wp.tile([128, 512], dt.float32)
                    nc.vector.tensor_copy(out=ot[:, :nt], in_=op_ps[:, :nt])
                    nc.sync.dma_start(
                        out=out[dh * 128:(dh + 1) * 128, n0:n0 + nt],
                        in_=ot[:, :nt])
    nc.compile()
    return nc


def _balance(deg, caps):
    """Assign NC nodes to NB blocks, balancing all 2P per-metapath lo/hi
    in-degree sums simultaneously (greedy, heaviest node first).
    deg: [2P, NC] float (rows pre-scaled so equal values = equal pressure).
    Returns (assign [NC], per-dim block loads [NB, 2P])."""
    order = np.argsort(-deg.sum(axis=0), kind="stable")
    loads = np.zeros((NB, deg.shape[0]), dtype=np.float64)
    counts = np.zeros(NB, dtype=np.int64)
    assign = np.empty(NC, dtype=np.int64)
    for n in order:
        feas = counts < caps
        newmax = np.where(feas[:, None], loads + deg[:, n], np.inf).max(axis=1)
        b = int(np.argmin(newmax))
        assign[n] = b
        loads[b] += deg[:, n]
        counts[b] += 1
    return assign, loads


def _prep_core(src_p, dst_p, base, KLO, KHI, blk_of, pos_of):
    """Per-core, per-metapath edge slot arrays. Edge slots within a block:
    lo edges (src < HLO) in chunks [0, KLO), hi edges in chunks [KLO, K).
    Pad slots gather row 0 and carry dst=200 (matches no iota lane).
    Returns (idx16 [128, NB*K*8] int16 (16-row wrapped, replicated x8),
    dstw [128, NB*K] bf16)."""
    K = KLO + KHI
    m = (dst_p >= base) & (dst_p < base + NC)
    s, d = src_p[m], dst_p[m] - base
    blk = blk_of[d]
    hi = (s >= HLO).astype(np.int64)
    order = np.lexsort((hi, blk))
    s, d, blk, hi = s[order], d[order], blk[order], hi[order]
    # position within (block, segment)
    seg = blk * 2 + hi
    cnt = np.bincount(seg, minlength=NB * 2)
    start = np.concatenate([[0], np.cumsum(cnt)])[:-1]
    pos = np.arange(len(d)) - start[seg]
    slot = blk * (K * 128) + hi * (KLO * 128) + pos
    si = np.zeros(NB * K * 128, dtype=np.int16)
    df = np.full(NB * K * 128, 200.0, dtype=np.float32)
    si[slot] = (s - hi * HLO).astype(np.int16)
    df[slot] = pos_of[d]
    # wrap idx: per block, per segment: j-th gathered -> idx[j%16, j//16]
    sib = si.reshape(NB, K * 128)
    lo_w = sib[:, :KLO * 128].reshape(NB, KLO * 8, 16).transpose(0, 2, 1)
    hi_w = sib[:, KLO * 128:].reshape(NB, KHI * 8, 16).transpose(0, 2, 1)
    idx16 = np.concatenate([lo_w, hi_w], axis=2)          # [NB, 16, K*8]
    idxr = np.tile(idx16, (1, 8, 1))                      # [NB, 128, K*8]
    dstr = df.reshape(NB, K, 128).transpose(0, 2, 1)      # [NB, 128, K]
    return (idxr.transpose(1, 0, 2).reshape(128, NB * K * 8),
            np.ascontiguousarray(dstr.transpose(1, 0, 2).reshape(
                128, NB * K)).astype(ml_dtypes.bfloat16))


def kernel(h, src, dst, W_gc, b_gc, W1, b1, w2):
    h = np.asarray(h, dtype=np.float32)
    src = np.asarray(src)
    dst = np.asarray(dst)

    # separable symmetric normalization: fold rsqrt(deg_out) into
    # per-metapath scaled bf16 copies of h; rsqrt(deg_in) goes to bvec
    h_los, h_his, rs_in = [], [], []
    for p in range(P):
        deg_out = np.clip(np.bincount(src[p], minlength=N), 1, None)
        deg_in = np.clip(np.bincount(dst[p], minlength=N), 1, None)
        ha = (h / np.sqrt(deg_out)[:, None]).astype(ml_dtypes.bfloat16)
        h_los.append(np.ascontiguousarray(ha[:HLO]))
        h_his.append(np.ascontiguousarray(ha[HLO:]))
        rs_in.append((1.0 / np.sqrt(deg_in)).astype(np.float32))

    # Balance nodes into blocks per core so the per-block lo/hi segment
    # sizes (and hence KLO/KHI, the uniform chunks-per-block) are minimal.
    # The 22 pad slots stay at the tail of the last block (caps 48x128+106),
    # keeping real nodes in slots [0, NC) for the on-device score masking.
    caps = np.full(NB, 128, dtype=np.int64)
    caps[NB - 1] = NC - (NB - 1) * 128
    scale = np.empty(2 * P, dtype=np.float64)
    scale[0::2] = 1.0                      # lo rows
    scale[1::2] = HLO / HHI                # hi rows get upscaled pressure
    blk_of, pos_of = [], []
    max_lo, max_hi = 0.0, 0.0
    for c in range(CORES):
        base = c * NC
        degs = []
        for p in range(P):
            m = (dst[p] >= base) & (dst[p] < base + NC)
            dl = dst[p][m] - base
            slo = src[p][m] < HLO
            degs.append(np.bincount(dl[slo], minlength=NC))
            degs.append(np.bincount(dl[~slo], minlength=NC))
        deg = np.stack(degs).astype(np.float64) * scale[:, None]
        assign, loads = _balance(deg, caps)
        raw = loads / scale[None, :]
        max_lo = max(max_lo, raw[:, 0::2].max())
        max_hi = max(max_hi, raw[:, 1::2].max())
        order = np.argsort(assign, kind="stable")
        pos = np.empty(NC, dtype=np.int64)
        starts = np.concatenate([[0], np.cumsum(np.bincount(assign,
                                                            minlength=NB))])
        pos[order] = np.arange(NC) - starts[assign[order]]
        blk_of.append(assign)
        pos_of.append(pos.astype(np.float32))
    KLO = (int(round(max_lo)) + 127) // 128
    KHI = (int(round(max_hi)) + 127) // 128

    if (KLO, KHI) not in _nc_cache:
        _nc_cache[(KLO, KHI)] = _build(KLO, KHI)
    nc = _nc_cache[(KLO, KHI)]

    bgc_arr = np.zeros((128, P * DH), dtype=np.float32)
    for p in range(P):
        for dh in range(DH):
            bgc_arr[:, p * DH + dh] = b_gc[p, dh * 128:(dh + 1) * 128]

    in_maps = []
    for c in range(CORES):
        base = c * NC
        idxs, dsts = [], []
        # node id at (block, pos) slot for this core (tail pads -> node 0)
        node_of = np.zeros(NCP, dtype=np.int64)
        node_of[blk_of[c] * 128 + pos_of[c].astype(np.int64)] = \
            np.arange(NC) + base
        bvec_arr = np.ones((128, P * NB), dtype=np.float32)
        for p in range(P):
            idx16, dstw = _prep_core(src[p], dst[p], base,
                                     KLO, KHI, blk_of[c], pos_of[c])
            idxs.append(idx16)
            dsts.append(dstw)
            bvec_arr[:, p * NB:(p + 1) * NB] = \
                rs_in[p][node_of].reshape(NB, 128).T
        im = {
            "gidx": np.ascontiguousarray(np.concatenate(idxs, axis=1)),
            "dstf": np.ascontiguousarray(np.concatenate(dsts, axis=1)),
            "bvec": bvec_arr,
            "Wgc": np.ascontiguousarray(W_gc, dtype=np.float32),
            "bgc": bgc_arr,
            "W1": np.ascontiguousarray(W1, dtype=np.float32),
            "b1": np.asarray(b1, dtype=np.float32).reshape(SEM_H, 1),
            "w2": np.asarray(w2, dtype=np.float32).reshape(SEM_H, 1),
        }
        for p in range(P):
            im[f"hlo{p}"] = h_los[p]
            im[f"hhi{p}"] = h_his[p]
        in_maps.append(im)

    global _last_in_maps
    _last_in_maps = in_maps
    res = run_bass_kernel_spmd(nc, in_maps, list(range(CORES))).results
    out = np.empty((N, D), dtype=np.float32)
    for c in range(CORES):
        slot = blk_of[c] * 128 + pos_of[c].astype(np.int64)
        out[c * NC:(c + 1) * NC] = res[c]["out"][:, slot].T
    return out
